# revision 25
# baseline (speedup 1.0000x reference)
"""Trainium2 Bass kernel for ClassicalReconstructionHydraSSMCore.

Quantum statevector simulation: batch 8192, 10 qubits, three circuits
(forward/backward/diagonal), combine + normalize + Pauli X/Y/Z measure.

Sharding: pure data parallel over batch across 8 cores (1024 each).
Per-core layout: batch on partitions (8 tiles of 128), state on free dim.

v2: f16 state planes (fp32 trig/coeff scalars). Rotations = 4 whole-state
tensor_scalar products (DVE 4x / ACT / Pool per plan tables) + 9 f16
tensor_tensor add/subs on DVE (2x mode). CRX uses baseline-proven views
with engine-cycled products. Gate emission is interleaved round-robin
across the three circuits and scratch is double-buffered across batch
tiles for cross-engine overlap. Measurement: X via sum-square identity
(|p0+p1|^2 - S) on ACT Square-accumulate; Y/Z/S via DVE stt-accumulate.
Note: tensor_tensor_reduce crashes the exec unit on this HW path; Pool
scalar_tensor_tensor is rejected by walrus codegen - avoid both.
"""

import numpy as np

import concourse.bass as bass
import concourse.tile as tile
from concourse import bacc, mybir

F32 = mybir.dt.float32
F16 = mybir.dt.float16
AOT = mybir.AluOpType
ACTF = mybir.ActivationFunctionType

NQ = 10
DIM = 1 << NQ          # 1024
HD = DIM // 2          # 512
QT = DIM // 4          # 256
P = 128
N_CORES = 8
B_CORE = 1024
NT = B_CORE // P       # 8 tiles per core
PI_2 = float(np.pi / 2)

FWD, BWD, DIAG = 0, 1, 2


def _wire(c, g):
    return (NQ - 1 - g) if c == BWD else g


def _ring_gates(c, L):
    """Time-ordered entangler list [(ctrl, tgt, col)] for circuit c, layer L."""
    base = 100 * c + 50 * L
    out = []
    if c in (FWD, DIAG):
        for k in range(NQ):
            out.append((k, (k + 1) % NQ, base + 30 + k))
        for k in range(NQ):
            i = NQ - 1 - k
            out.append((i, (i - 1) % NQ, base + 40 + k))
    else:
        for k in range(NQ):
            i = NQ - 1 - k
            out.append((i, (i - 1) % NQ, base + 30 + k))
        for k in range(NQ):
            out.append((k, (k + 1) % NQ, base + 40 + k))
    return out


def _wire_views(plane, w):
    inner = 1 << (NQ - 1 - w)
    outer = HD // inner
    v = plane.rearrange("p (o t i) -> p o t i", o=outer, t=2, i=inner)
    return v[:, :, 0, :], v[:, :, 1, :]


def _qviews2(plane, ctrl, tgt):
    """ctrl=1 quarters (tgt=0, tgt=1) of a (128,1024) plane; <=2 free dims."""
    hi, lo = min(ctrl, tgt), max(ctrl, tgt)
    if lo - hi == 1:
        a = 1 << hi
        z = 1 << (NQ - 2 - hi)
        v = plane.rearrange("p (a x y z) -> p a x y z", a=a, x=2, y=2, z=z)
        if ctrl < tgt:
            return v[:, :, 1, 0, :], v[:, :, 1, 1, :]
        return v[:, :, 0, 1, :], v[:, :, 1, 1, :]
    assert hi == 0 and lo == NQ - 1
    v = plane.rearrange("p (x b y) -> p x b y", x=2, b=DIM // 4, y=2)
    if ctrl == 0:
        return v[:, 1, :, 0], v[:, 1, :, 1]
    return v[:, 0, :, 1], v[:, 1, :, 1]


def emit_core_kernel(nc, tc, ins, outs, n_tiles=NT):
    ang_d = ins["input_angles"]
    par_d = [ins["forward_params"], ins["backward_params"], ins["diagonal_params"]]
    dth_d = ins["dth"]
    cf_d = ins["cf"]
    out_d = outs["out"]

    # ---- engine helpers ----
    ts_d = nc.vector.tensor_scalar_mul
    ts_a = nc.scalar.mul
    ts_p = nc.gpsimd.tensor_scalar_mul
    tt_d = lambda o, a, b, op=AOT.add: nc.vector.tensor_tensor(o, a, b, op=op)
    tt_p = lambda o, a, b, op=AOT.add: nc.gpsimd.tensor_tensor(o, a, b, op=op)
    stt_d = lambda o, i0, s, i1: nc.vector.scalar_tensor_tensor(
        o, i0, s, i1, op0=AOT.mult, op1=AOT.add
    )
    stt_p = lambda o, i0, s, i1: nc.gpsimd.scalar_tensor_tensor(
        o, i0, s, i1, op0=AOT.mult, op1=AOT.add
    )
    TS = {"d": ts_d, "a": ts_a, "p": ts_p}
    TT = {"d": tt_d, "p": tt_p}

    # per-site engine plans, cycled per gate instance (tuning knobs)
    # rot: prods = engines for the 4 whole-state (2048) products p,r,q,s;
    #      all adds on DVE (f16 2x tt).
    ROT_PLANS = [
        ("a", "p", "a", "a"),
        ("a", "p", "a", "d"),
        ("a", "d", "a", "p"),
    ]
    # crx: pr = 2 product ts (512); ad = 2 adds:
    #   "s" = fused stt on DVE, "ad"/"pd" = cc-product on ACT/Pool + DVE tt
    CRX_PLANS = [
        dict(pr4=("p", "a", "a", "p"), ad=("fa", "s")),
        dict(pr4=("a", "p", "p", "a"), ad=("s", "fp")),
        dict(pr4=("p", "a", "a", "p"), ad=("s", "fa")),
        dict(pr4=("a", "p", "p", "a"), ad=("fp", "s")),
    ]
    counters = {"rot": 0, "crx": 0}

    with (
        tc.tile_pool(name="const", bufs=1) as cpool,
        tc.tile_pool(name="work", bufs=2) as pool,
        tc.tile_pool(name="tmps", bufs=1) as tpool,
    ):
        cf_t = cpool.tile([P, 16], F32)
        nc.sync.dma_start(cf_t[:, 0 : cf_d.shape[1]], cf_d[:])
        pi2 = cpool.tile([P, 1], F32)
        nc.gpsimd.memset(pi2[:], PI_2)
        pi2c = pi2[:, 0:1]

        for t in range(n_tiles):
            r0, r1 = t * P, (t + 1) * P
            # ---- loads ----
            par = pool.tile([P, 300], F32, tag="par")
            for c in range(3):
                nc.sync.dma_start(par[:, 100 * c : 100 * (c + 1)], par_d[c][r0:r1, :])
            ang = pool.tile([P, NQ], F32, tag="ang")
            nc.sync.dma_start(ang[:], ang_d[r0:r1, :])
            dth = pool.tile([P, 1], F32, tag="dth")
            nc.sync.dma_start(dth[:], dth_d[r0:r1, :])

            # ---- trig ----
            # ScalarE Sin covers [-pi, pi]; quarter angles:
            # u = sin(h/2), w = cos(h/2); sin(h)=2uw, cos(h)=1-2u^2.
            ch = pool.tile([P, 300], F32, tag="ch")
            sh = pool.tile([P, 300], F32, tag="sh")
            nsh = pool.tile([P, 300], F32, tag="nsh")
            trA = pool.tile([P, 100], F32, tag="trA")
            trB = pool.tile([P, 100], F32, tag="trB")

            def emit_trig(dst_s, dst_c, src, scale, scrA, scrB):
                nc.scalar.activation(dst_s, src, ACTF.Sin, scale=scale)
                nc.scalar.activation(dst_c, src, ACTF.Sin, scale=scale, bias=pi2c)
                tt_p(scrA, dst_s, dst_c, op=AOT.mult)
                tt_p(scrB, dst_s, dst_s, op=AOT.mult)
                nc.gpsimd.tensor_scalar_mul(dst_s, scrA, 2.0)
                nc.gpsimd.tensor_scalar(dst_c, scrB, -2.0, 1.0, op0=AOT.mult, op1=AOT.add)

            for c in range(3):
                src = par[:, 100 * c : 100 * (c + 1)]
                dst_s = sh[:, 100 * c : 100 * (c + 1)]
                dst_c = ch[:, 100 * c : 100 * (c + 1)]
                if c == DIAG:
                    emit_trig(dst_s, dst_c, src, 0.25, trA[:], trB[:])
                else:
                    emit_trig(dst_s, dst_c, src, dth[:, 0:1], trA[:], trB[:])
                    # fix CRX cols (30-49, 80-99): no dt factor
                    lx = lambda ap: ap.rearrange("p (l x) -> p l x", l=2, x=50)[:, :, 30:50]
                    emit_trig(
                        lx(dst_s), lx(dst_c), lx(src), 0.25,
                        trA[:, 0:40].rearrange("p (l x) -> p l x", l=2, x=20),
                        trB[:, 0:40].rearrange("p (l x) -> p l x", l=2, x=20),
                    )
            nc.vector.tensor_scalar_mul(nsh[:], sh[:], -1.0)

            angc = pool.tile([P, NQ], F32, tag="angc")
            angs = pool.tile([P, NQ], F32, tag="angs")
            emit_trig(angs[:], angc[:], ang[:], 0.25, trA[:, 0:NQ], trB[:, 0:NQ])
            a3c = pool.tile([P, 30], F32, tag="a3c")
            a3s = pool.tile([P, 30], F32, tag="a3s")
            nc.scalar.copy(a3c[:, 0:10], angc[:])
            nc.scalar.copy(a3c[:, 10:20], angc[:, ::-1])
            nc.scalar.copy(a3c[:, 20:30], angc[:])
            nc.scalar.copy(a3s[:, 0:10], angs[:])
            nc.scalar.copy(a3s[:, 10:20], angs[:, ::-1])
            nc.scalar.copy(a3s[:, 20:30], angs[:])

            # ---- u-coefficients per layer: p,q,nq,r,nr,s,ns (128,30) ----
            ch3 = ch[:].rearrange("p (c x) -> p c x", c=3, x=100)
            sh3 = sh[:].rearrange("p (c x) -> p c x", c=3, x=100)
            U = []
            m1 = pool.tile([P, 30], F32, tag="m1")
            m2 = pool.tile([P, 30], F32, tag="m2")
            m3 = pool.tile([P, 30], F32, tag="m3")
            m4 = pool.tile([P, 30], F32, tag="m4")
            w1 = pool.tile([P, 30], F32, tag="w1")
            w2 = pool.tile([P, 30], F32, tag="w2")
            V = lambda tl: tl[:].rearrange("p (c g) -> p c g", c=3, g=10)
            for L in range(2):
                ca = ch3[:, :, 50 * L : 50 * L + 30 : 3]
                cb = ch3[:, :, 50 * L + 1 : 50 * L + 30 : 3]
                cg = ch3[:, :, 50 * L + 2 : 50 * L + 30 : 3]
                sa = sh3[:, :, 50 * L : 50 * L + 30 : 3]
                sb = sh3[:, :, 50 * L + 1 : 50 * L + 30 : 3]
                sg = sh3[:, :, 50 * L + 2 : 50 * L + 30 : 3]
                u = {
                    k: pool.tile([P, 30], F32, tag=f"u{k}{L}", name=f"u{k}{L}")
                    for k in ("p", "q", "nq", "r", "nr", "s", "ns")
                }
                tt_p(V(m1), cb, ca, op=AOT.mult)
                tt_p(V(m2), sb, sa, op=AOT.mult)
                tt_p(V(m3), sb, ca, op=AOT.mult)
                tt_p(V(m4), cb, sa, op=AOT.mult)
                tt_p(V(w1), cg, V(m1), op=AOT.mult)
                tt_p(V(w2), sg, V(m2), op=AOT.mult)
                tt_p(V(u["p"]), V(w1), V(w2), op=AOT.add)
                tt_p(V(w1), cg, V(m2), op=AOT.mult)
                tt_p(V(w2), sg, V(m1), op=AOT.mult)
                tt_p(V(u["q"]), V(w1), V(w2), op=AOT.subtract)
                tt_p(V(w1), cg, V(m3), op=AOT.mult)
                tt_p(V(w2), sg, V(m4), op=AOT.mult)
                tt_p(V(u["nr"]), V(w1), V(w2), op=AOT.add)
                tt_p(V(w1), sg, V(m3), op=AOT.mult)
                tt_p(V(w2), cg, V(m4), op=AOT.mult)
                tt_p(V(u["s"]), V(w1), V(w2), op=AOT.subtract)
                nc.gpsimd.tensor_scalar_mul(u["nq"][:], u["q"][:], -1.0)
                nc.gpsimd.tensor_scalar_mul(u["r"][:], u["nr"][:], -1.0)
                nc.gpsimd.tensor_scalar_mul(u["ns"][:], u["s"][:], -1.0)
                U.append(u)

            # ---- v vectors: layer-0 rotations folded into init ----
            u0 = U[0]
            v0r = pool.tile([P, 30], F32, tag="v0r")
            v0i = pool.tile([P, 30], F32, tag="v0i")
            v1r = pool.tile([P, 30], F32, tag="v1r")
            v1i = pool.tile([P, 30], F32, tag="v1i")
            for dst, t1, t2 in (
                (v0r, ("p", a3c), ("r", a3s)),
                (v0i, ("q", a3c), ("s", a3s)),
                (v1r, ("nr", a3c), ("p", a3s)),
                (v1i, ("s", a3c), ("nq", a3s)),
            ):
                tt_p(w1[:], u0[t1[0]][:], t1[1][:], op=AOT.mult)
                tt_p(w2[:], u0[t2[0]][:], t2[1][:], op=AOT.mult)
                tt_p(dst[:], w1[:], w2[:], op=AOT.add)

            # ---- per-circuit state (f16) + scratch ----
            st = [pool.tile([P, 2 * DIM], F16, tag=f"st{c}", name=f"st{c}") for c in range(3)]
            tmp = [
                [
                    pool.tile([P, 2 * DIM], F16, tag=f"tmp{c}_{k}", name=f"tmp{c}_{k}")
                    for k in range(4)
                ]
                for c in range(3)
            ]
            ab = [
                [pool.tile([P, 32], F16, tag=f"ab{c}_{k}", name=f"ab{c}_{k}") for k in range(8)]
                for c in range(3)
            ]
            arX = pool.tile([P, DIM], F16, tag="arX")
            aiX = pool.tile([P, DIM], F16, tag="aiX")

            def expand(c, bufs, wires, col_of):
                """Log-doubling product build over `wires` into bufs (r,i,r2,i2)."""
                br, bi, br2, bi2 = bufs
                j0 = col_of(wires[0])
                nc.vector.tensor_copy(br[:, 0:1], v0r[:, j0 : j0 + 1])
                nc.vector.tensor_copy(br[:, 1:2], v1r[:, j0 : j0 + 1])
                nc.vector.tensor_copy(bi[:, 0:1], v0i[:, j0 : j0 + 1])
                nc.vector.tensor_copy(bi[:, 1:2], v1i[:, j0 : j0 + 1])
                width = 2
                cur_r, cur_i, oth_r, oth_i = br, bi, br2, bi2
                for w in wires[1:]:
                    j = col_of(w)
                    c0r, c0i = v0r[:, j : j + 1], v0i[:, j : j + 1]
                    c1r, c1i = v1r[:, j : j + 1], v1i[:, j : j + 1]
                    old_r, old_i = cur_r[:, 0:width], cur_i[:, 0:width]
                    nw = 2 * width
                    nr_v = oth_r[:, 0:nw].rearrange("p (w t) -> p w t", w=width, t=2)
                    ni_v = oth_i[:, 0:nw].rearrange("p (w t) -> p w t", w=width, t=2)
                    tt0 = tmp[c][0][:, 0:width]
                    tt1 = tmp[c][1][:, 0:width]
                    tt2 = tmp[c][2][:, 0:width]
                    tt3 = tmp[c][3][:, 0:width]
                    # (r+ii)(cr+ici): re = r*cr - i*ci ; im = r*ci + i*cr
                    ts_p(tt0, old_i, c0i)
                    nc.vector.scalar_tensor_tensor(
                        nr_v[:, :, 0], old_r, c0r, tt0, op0=AOT.mult, op1=AOT.subtract
                    )
                    ts_p(tt1, old_i, c0r)
                    stt_d(ni_v[:, :, 0], old_r, c0i, tt1)
                    ts_p(tt2, old_i, c1i)
                    nc.vector.scalar_tensor_tensor(
                        nr_v[:, :, 1], old_r, c1r, tt2, op0=AOT.mult, op1=AOT.subtract
                    )
                    ts_p(tt3, old_i, c1r)
                    stt_d(ni_v[:, :, 1], old_r, c1i, tt3)
                    cur_r, oth_r = oth_r, cur_r
                    cur_i, oth_i = oth_i, cur_i
                    width = nw
                return cur_r, cur_i

            def emit_rot(c, stt_c, w, u, j):
                """General SU(2) gate on wire w, full state.

                4 whole-state (2048) scalar products + 9 f16 tt add/subs:
                  a0' = p*a0 + r*a1 - (q*a0i + s*a1i | -(q*a0r + s*a1r))
                  a1' = p*a1 - r*a0 + (q*a1i - s*a0i | s*a0r - q*a1r)
                """
                plan = ROT_PLANS[counters["rot"] % len(ROT_PLANS)]
                counters["rot"] += 1
                sp = u["p"][:, j : j + 1]
                sq = u["q"][:, j : j + 1]
                sr = u["r"][:, j : j + 1]
                ss = u["s"][:, j : j + 1]
                re, im = stt_c[:, 0:DIM], stt_c[:, DIM : 2 * DIM]
                inner = 1 << (NQ - 1 - w)
                outer = HD // inner
                m = 2 * outer
                fm = stt_c[:].rearrange("p (m t i) -> p m t i", m=m, t=2, i=inner)
                a0m = fm[:, :, 0, :]
                a1m = fm[:, :, 1, :]
                a0r, a1r = _wire_views(re, w)
                a0i, a1i = _wire_views(im, w)
                TW, RP, QF, SF = tmp[c][0:4]
                tv = lambda tp, tbit: tp[:].rearrange(
                    "p (m t i) -> p m t i", m=m, t=2, i=inner
                )[:, :, tbit, :]
                pv = lambda tp, off, tbit: tp[:, off : off + DIM].rearrange(
                    "p (o t i) -> p o t i", o=outer, t=2, i=inner
                )[:, :, tbit, :]
                hv = lambda tp, h: tp[:, h * HD : (h + 1) * HD].rearrange(
                    "p (o i) -> p o i", o=outer, i=inner
                )
                E = [TS[k] for k in plan]
                # whole-state products (2048 each)
                E[0](TW[:], stt_c[:], sp)
                E[1](RP[:], stt_c[:], sr)
                E[2](QF[:], stt_c[:], sq)   # [q*re | q*im]
                E[3](SF[:], stt_c[:], ss)   # [s*re | s*im]
                # aligned into state (reads TW/RP only)
                tt_d(a0m, tv(TW, 0), tv(RP, 1), op=AOT.add)
                tt_d(a1m, tv(TW, 1), tv(RP, 0), op=AOT.subtract)
                # swapped partials into TW halves (TW dead after aligned adds)
                # T1n = s*a1i + q*a0i ; T1i = q*a0r + s*a1r
                # T3r = q*a1i - s*a0i ; T3i = s*a0r - q*a1r
                tt_d(hv(TW, 0), pv(SF, DIM, 1), pv(QF, DIM, 0), op=AOT.add)
                tt_d(hv(TW, 1), pv(QF, 0, 0), pv(SF, 0, 1), op=AOT.add)
                tt_d(hv(TW, 2), pv(QF, DIM, 1), pv(SF, DIM, 0), op=AOT.subtract)
                tt_d(hv(TW, 3), pv(SF, 0, 0), pv(QF, 0, 1), op=AOT.subtract)
                # finals: a0 per-half (sign differs), a1 merged
                tt_d(a0r, a0r, hv(TW, 0), op=AOT.subtract)
                tt_d(a0i, a0i, hv(TW, 1), op=AOT.add)
                t3m = TW[:, DIM : 2 * DIM].rearrange("p (m i) -> p m i", m=m, i=inner)
                tt_d(a1m, a1m, t3m, op=AOT.add)

            def emit_crx(c, stt_c, ctrl, tgt, col):
                """CRX via baseline-proven views: 4 quarter products + 2
                merged fused adds (stt on DVE)."""
                plan = CRX_PLANS[counters["crx"] % len(CRX_PLANS)]
                counters["crx"] += 1
                cc = ch[:, col : col + 1]
                ss = sh[:, col : col + 1]
                ns = nsh[:, col : col + 1]
                re, im = stt_c[:, 0:DIM], stt_c[:, DIM : 2 * DIM]
                q0r, q1r = _qviews2(re, ctrl, tgt)
                q0i, q1i = _qviews2(im, ctrl, tgt)
                hi, lo = min(ctrl, tgt), max(ctrl, tgt)
                t23 = tmp[c][2][:, 0:HD]
                ee = tmp[c][2][:, HD:DIM]
                E = [TS[k] for k in plan["pr4"]]
                if lo - hi == 1:
                    a = 1 << hi
                    z = 1 << (NQ - 2 - hi)
                    ma = 2 * a
                    fm = stt_c[:].rearrange(
                        "p (ma x y z) -> p ma x y z", ma=ma, x=2, y=2, z=z
                    )
                    if ctrl < tgt:
                        q0m = fm[:, :, 1, 0, :]
                        q1m = fm[:, :, 1, 1, :]
                    else:
                        q0m = fm[:, :, 0, 1, :]
                        q1m = fm[:, :, 1, 1, :]
                    pvm = lambda tp: tp.rearrange("p (ma z) -> p ma z", ma=ma, z=z)
                    hv2 = lambda tp, h: tp[:, h * QT : (h + 1) * QT].rearrange(
                        "p (a z) -> p a z", a=a, z=z
                    )
                    E[0](hv2(t23, 0), q0i, ss)
                    E[1](hv2(t23, 1), q0r, ns)
                    E[2](hv2(ee, 0), q1i, ss)
                    E[3](hv2(ee, 1), q1r, ns)

                    def fadd2(qm, addend, mode, off):
                        if mode == "s":
                            stt_d(qm, qm, cc, addend)
                            return
                        pbv = tmp[c][3][:, off : off + HD].rearrange(
                            "p (ma z) -> p ma z", ma=ma, z=z
                        )
                        TS["a" if mode == "fa" else "p"](pbv, qm, cc)
                        tt_d(qm, pbv, addend, op=AOT.add)

                    fadd2(q0m, pvm(ee), plan["ad"][0], 0)
                    fadd2(q1m, pvm(t23), plan["ad"][1], HD)
                else:
                    t0 = t23[:, 0:QT]
                    t1 = t23[:, QT:HD]
                    t2 = ee[:, 0:QT]
                    t3 = ee[:, QT:HD]
                    E[0](t0, q0r, cc)
                    E[1](t1, q0i, cc)
                    E[2](t2, q0i, ss)
                    E[3](t3, q0r, ns)
                    stt_d(q0r, q1i, ss, t0)
                    stt_d(q0i, q1r, ns, t1)
                    stt_d(q1r, q1r, cc, t2)
                    stt_d(q1i, q1i, cc, t3)

            for c in range(3):
                col_of = lambda w, c=c: 10 * c + (w if c != BWD else NQ - 1 - w)
                ar, ai = expand(c, ab[c][0:4], list(range(5)), col_of)
                br_, bi_ = expand(c, ab[c][4:8], list(range(5, NQ)), col_of)
                sre = st[c][:, 0:DIM].rearrange("p (i j) -> p i j", i=32, j=32)
                sim_ = st[c][:, DIM : 2 * DIM].rearrange("p (i j) -> p i j", i=32, j=32)
                arXv = arX[:].rearrange("p (i j) -> p i j", i=32, j=32)
                aiXv = aiX[:].rearrange("p (i j) -> p i j", i=32, j=32)
                arb = ar[:].broadcast_to([P, 32, 32])
                aib = ai[:].broadcast_to([P, 32, 32])
                brb = br_[:].broadcast_to([P, 32, 32]).transpose([0, 2, 1])
                bib = bi_[:].broadcast_to([P, 32, 32]).transpose([0, 2, 1])
                # materialize the stride-0-innermost operands once -> packed TTs
                nc.vector.tensor_copy(arXv, arb)
                nc.vector.tensor_copy(aiXv, aib)
                tt_d(sre, arXv, brb, op=AOT.mult)
                tt_p(sim_, aiXv, bib, op=AOT.mult)
                tt_d(sre, sre, sim_, op=AOT.subtract)
                tt_d(sim_, arXv, bib, op=AOT.mult)
                scrv = tmp[c][2][:, 0:DIM].rearrange("p (i j) -> p i j", i=32, j=32)
                tt_p(scrv, aiXv, brb, op=AOT.mult)
                tt_d(sim_, sim_, scrv, op=AOT.add)

            # gates, interleaved round-robin across circuits so the three
            # independent chains pack the engines
            ring0 = [_ring_gates(c, 0) for c in range(3)]
            ring1 = [_ring_gates(c, 1) for c in range(3)]
            for k in range(2 * NQ):
                for c in range(3):
                    ctrl, tgt, col = ring0[c][k]
                    emit_crx(c, st[c], ctrl, tgt, col)
            for g in range(NQ):
                for c in range(3):
                    emit_rot(c, st[c], _wire(c, g), U[1], 10 * c + g)
            for k in range(2 * NQ):
                for c in range(3):
                    ctrl, tgt, col = ring1[c][k]
                    emit_crx(c, st[c], ctrl, tgt, col)

            # ---- combine: acc = c1*psi1 + c2*psi2 + c3*psi3 (f16) ----
            acc = pool.tile([P, 2 * DIM], F16, tag="acc")
            s1t = pool.tile([P, 2 * DIM], F16, tag="s1t")
            cfc = lambda k: cf_t[:, k : k + 1]
            # cf cols: [c1r, c1i, nc1i, c2r, c2i, nc2i, c3r, c3i, nc3i]
            for k in range(3):
                stk = st[k]
                swapped = stk[:].rearrange("p (pl x) -> p pl x", pl=2, x=DIM)[:, ::-1, :]
                if k == 0:
                    ts_d(acc[:], stk[:], cfc(0))
                    ts_d(s1t[:], stk[:], cfc(1))
                else:
                    stt_d(acc[:], stk[:], cfc(3 * k), acc[:])
                    ts_d(s1t[:], stk[:], cfc(3 * k + 1))
                # acc_re -= c_i*st_im ; acc_im += c_i*st_re
                tt_d(acc[:, 0:DIM], acc[:, 0:DIM], s1t[:, DIM : 2 * DIM], op=AOT.subtract)
                tt_d(acc[:, DIM : 2 * DIM], acc[:, DIM : 2 * DIM], s1t[:, 0:DIM], op=AOT.add)

            # ---- measure ----
            xw = pool.tile([P, NQ], F32, tag="xw")
            yw = pool.tile([P, NQ], F32, tag="yw")
            zw = pool.tile([P, NQ], F32, tag="zw")
            scol = pool.tile([P, 8], F32, tag="scol")
            mscr = pool.tile([P, DIM], F16, tag="mscr")
            mx = pool.tile([P, DIM], F16, tag="mx")
            mY = pool.tile([P, 2 * DIM], F16, tag="mY")

            # S = sum |acc|^2  (proven stt-accumulate pattern)
            nc.vector.scalar_tensor_tensor(
                mY[:], acc[:], 0.0, acc[:], op0=AOT.bypass, op1=AOT.mult,
                accum_out=scol[:, 0:1],
            )
            # mY = [acc_im | -acc_re]  for the Y cross term
            nc.vector.tensor_copy(mY[:, 0:DIM], acc[:, DIM : 2 * DIM])
            ts_d(mY[:, DIM : 2 * DIM], acc[:, 0:DIM], -1.0)

            for w in range(NQ):
                inner = 1 << (NQ - 1 - w)
                m = 2 * (HD // inner)
                fv = acc[:].rearrange("p (m t i) -> p m t i", m=m, t=2, i=inner)
                p0 = fv[:, :, 0, :]
                p1 = fv[:, :, 1, :]
                fy = mY[:].rearrange("p (m t i) -> p m t i", m=m, t=2, i=inner)
                p1y = fy[:, :, 1, :]
                ms = mscr[:].rearrange("p (m i) -> p m i", m=m, i=inner)
                mxv = mx[:].rearrange("p (m i) -> p m i", m=m, i=inner)
                tt_d(mxv, p0, p1, op=AOT.add)
                nc.scalar.activation(
                    mxv, mxv, ACTF.Square, accum_out=xw[:, w : w + 1]
                )
                nc.vector.scalar_tensor_tensor(
                    ms, p0, 0.0, p1y, op0=AOT.bypass, op1=AOT.mult,
                    accum_out=yw[:, w : w + 1],
                )
                nc.scalar.activation(
                    ms, p1, ACTF.Square, accum_out=zw[:, w : w + 1]
                )

            # inv = 1/(S + 1e-9); s2 = 2*inv; sn2 = -2*inv; sz = S*inv
            nc.vector.tensor_scalar(
                scol[:, 1:2], scol[:, 0:1], 1e-9, None, op0=AOT.add
            )
            nc.vector.reciprocal(scol[:, 2:3], scol[:, 1:2])
            nc.vector.tensor_scalar(scol[:, 3:4], scol[:, 2:3], 2.0, None, op0=AOT.mult)
            nc.vector.tensor_scalar(scol[:, 4:5], scol[:, 2:3], -2.0, None, op0=AOT.mult)
            tt_d(scol[:, 5:6], scol[:, 0:1], scol[:, 2:3], op=AOT.mult)
            nc.vector.tensor_scalar(scol[:, 6:7], scol[:, 5:6], -1.0, None, op0=AOT.mult)

            out30 = pool.tile([P, 30], F32, tag="out30")
            nc.vector.tensor_scalar(
                out30[:, 0:10], xw[:], scol[:, 2:3], scol[:, 6:7],
                op0=AOT.mult, op1=AOT.add,
            )
            ts_d(out30[:, 10:20], yw[:], scol[:, 3:4])
            szb = scol[:, 5:6].broadcast_to([P, 1, NQ])
            nc.vector.scalar_tensor_tensor(
                out30[:, 20:30].unsqueeze(1), zw[:].unsqueeze(1), scol[:, 4:5], szb,
                op0=AOT.mult, op1=AOT.add,
            )
            nc.sync.dma_start(out_d[r0:r1, :], out30[:])


def build_nc(n_tiles=NT, b_core=None):
    if b_core is None:
        b_core = n_tiles * P
    nc = bacc.Bacc("TRN2", target_bir_lowering=False)
    ins = {
        "input_angles": nc.dram_tensor("input_angles", [b_core, NQ], F32, kind="ExternalInput")[:],
        "forward_params": nc.dram_tensor("forward_params", [b_core, 100], F32, kind="ExternalInput")[:],
        "backward_params": nc.dram_tensor("backward_params", [b_core, 100], F32, kind="ExternalInput")[:],
        "diagonal_params": nc.dram_tensor("diagonal_params", [b_core, 100], F32, kind="ExternalInput")[:],
        "dth": nc.dram_tensor("dth", [b_core, 1], F32, kind="ExternalInput")[:],
        "cf": nc.dram_tensor("cf", [P, 9], F32, kind="ExternalInput")[:],
    }
    outs = {"out": nc.dram_tensor("out", [b_core, 30], F32, kind="ExternalOutput")[:]}
    with tile.TileContext(nc) as tc:
        emit_core_kernel(nc, tc, ins, outs, n_tiles=n_tiles)
    nc.compile()
    return nc


_NC_CACHE = {}


def _get_nc(n_tiles=NT):
    if n_tiles not in _NC_CACHE:
        _NC_CACHE[n_tiles] = build_nc(n_tiles)
    return _NC_CACHE[n_tiles]


def make_host_inputs(input_angles, forward_params, backward_params, diagonal_params,
                     dt_scale, alpha_real, alpha_imag, beta_real, beta_imag,
                     gamma_real, gamma_imag):
    """Host-side scalar prep shared by kernel() and tests."""
    al = complex(float(alpha_real), float(alpha_imag))
    be = complex(float(beta_real), float(beta_imag))
    ga = complex(float(gamma_real), float(gamma_imag))
    n = np.sqrt(abs(al) ** 2 + abs(be) ** 2 + abs(ga) ** 2 + 1e-9)
    cs = [al / n, be / n, ga / n]
    row = []
    for ck in cs:
        row += [ck.real, ck.imag, -ck.imag]
    cf = np.tile(np.asarray(row, np.float32), (P, 1))
    dth = (0.25 * np.asarray(dt_scale, np.float32)).reshape(-1, 1)
    return cf, dth


def kernel(**inputs):
    from concourse.bass_utils import run_bass_kernel_spmd

    cf, dth = make_host_inputs(**inputs)
    ang = np.ascontiguousarray(np.asarray(inputs["input_angles"], np.float32))
    pf = np.ascontiguousarray(np.asarray(inputs["forward_params"], np.float32))
    pb = np.ascontiguousarray(np.asarray(inputs["backward_params"], np.float32))
    pd = np.ascontiguousarray(np.asarray(inputs["diagonal_params"], np.float32))

    nc = _get_nc(NT)
    in_maps = []
    for c in range(N_CORES):
        r0, r1 = c * B_CORE, (c + 1) * B_CORE
        in_maps.append({
            "input_angles": ang[r0:r1],
            "forward_params": pf[r0:r1],
            "backward_params": pb[r0:r1],
            "diagonal_params": pd[r0:r1],
            "dth": np.ascontiguousarray(dth[r0:r1]),
            "cf": cf,
        })
    res = run_bass_kernel_spmd(nc, in_maps, core_ids=list(range(N_CORES)))
    out = np.concatenate([res.results[c]["out"] for c in range(N_CORES)], axis=0)
    return out.astype(np.float32)


# revision 27
# speedup vs baseline: 1.0079x; 1.0079x over previous
"""Trainium2 Bass kernel for ClassicalReconstructionHydraSSMCore.

Quantum statevector simulation: batch 8192, 10 qubits, three circuits
(forward/backward/diagonal), combine + normalize + Pauli X/Y/Z measure.

Sharding: pure data parallel over batch across 8 cores (1024 each).
Per-core layout: batch on partitions (8 tiles of 128), state on free dim.

v2: f16 state planes (fp32 trig/coeff scalars). Rotations = 4 whole-state
tensor_scalar products (DVE 4x / ACT / Pool per plan tables) + 9 f16
tensor_tensor add/subs on DVE (2x mode). CRX uses baseline-proven views
with engine-cycled products. Gate emission is interleaved round-robin
across the three circuits and scratch is double-buffered across batch
tiles for cross-engine overlap. Measurement: X via sum-square identity
(|p0+p1|^2 - S) on ACT Square-accumulate; Y/Z/S via DVE stt-accumulate.
Note: tensor_tensor_reduce crashes the exec unit on this HW path; Pool
scalar_tensor_tensor is rejected by walrus codegen - avoid both.
"""

import numpy as np

import concourse.bass as bass
import concourse.tile as tile
from concourse import bacc, mybir

F32 = mybir.dt.float32
F16 = mybir.dt.float16
AOT = mybir.AluOpType
ACTF = mybir.ActivationFunctionType

NQ = 10
DIM = 1 << NQ          # 1024
HD = DIM // 2          # 512
QT = DIM // 4          # 256
P = 128
N_CORES = 8
B_CORE = 1024
NT = B_CORE // P       # 8 tiles per core
PI_2 = float(np.pi / 2)

FWD, BWD, DIAG = 0, 1, 2


def _wire(c, g):
    return (NQ - 1 - g) if c == BWD else g


def _ring_gates(c, L):
    """Time-ordered entangler list [(ctrl, tgt, col)] for circuit c, layer L."""
    base = 100 * c + 50 * L
    out = []
    if c in (FWD, DIAG):
        for k in range(NQ):
            out.append((k, (k + 1) % NQ, base + 30 + k))
        for k in range(NQ):
            i = NQ - 1 - k
            out.append((i, (i - 1) % NQ, base + 40 + k))
    else:
        for k in range(NQ):
            i = NQ - 1 - k
            out.append((i, (i - 1) % NQ, base + 30 + k))
        for k in range(NQ):
            out.append((k, (k + 1) % NQ, base + 40 + k))
    return out


def _wire_views(plane, w):
    inner = 1 << (NQ - 1 - w)
    outer = HD // inner
    v = plane.rearrange("p (o t i) -> p o t i", o=outer, t=2, i=inner)
    return v[:, :, 0, :], v[:, :, 1, :]


def _qviews2(plane, ctrl, tgt):
    """ctrl=1 quarters (tgt=0, tgt=1) of a (128,1024) plane; <=2 free dims."""
    hi, lo = min(ctrl, tgt), max(ctrl, tgt)
    if lo - hi == 1:
        a = 1 << hi
        z = 1 << (NQ - 2 - hi)
        v = plane.rearrange("p (a x y z) -> p a x y z", a=a, x=2, y=2, z=z)
        if ctrl < tgt:
            return v[:, :, 1, 0, :], v[:, :, 1, 1, :]
        return v[:, :, 0, 1, :], v[:, :, 1, 1, :]
    assert hi == 0 and lo == NQ - 1
    v = plane.rearrange("p (x b y) -> p x b y", x=2, b=DIM // 4, y=2)
    if ctrl == 0:
        return v[:, 1, :, 0], v[:, 1, :, 1]
    return v[:, 0, :, 1], v[:, 1, :, 1]


def emit_core_kernel(nc, tc, ins, outs, n_tiles=NT):
    ang_d = ins["input_angles"]
    par_d = [ins["forward_params"], ins["backward_params"], ins["diagonal_params"]]
    dth_d = ins["dth"]
    cf_d = ins["cf"]
    out_d = outs["out"]

    # ---- engine helpers ----
    ts_d = nc.vector.tensor_scalar_mul
    ts_a = nc.scalar.mul
    ts_p = nc.gpsimd.tensor_scalar_mul
    tt_d = lambda o, a, b, op=AOT.add: nc.vector.tensor_tensor(o, a, b, op=op)
    tt_p = lambda o, a, b, op=AOT.add: nc.gpsimd.tensor_tensor(o, a, b, op=op)
    stt_d = lambda o, i0, s, i1: nc.vector.scalar_tensor_tensor(
        o, i0, s, i1, op0=AOT.mult, op1=AOT.add
    )
    stt_p = lambda o, i0, s, i1: nc.gpsimd.scalar_tensor_tensor(
        o, i0, s, i1, op0=AOT.mult, op1=AOT.add
    )
    TS = {"d": ts_d, "a": ts_a, "p": ts_p}
    TT = {"d": tt_d, "p": tt_p}

    # per-site engine plans, cycled per gate instance (tuning knobs)
    # rot: prods = engines for the 4 whole-state (2048) products p,r,q,s;
    #      all adds on DVE (f16 2x tt).
    ROT_PLANS = [
        ("a", "p", "a", "a"),
        ("a", "p", "a", "d"),
        ("a", "d", "a", "p"),
    ]
    # crx: pr = 2 product ts (512); ad = 2 adds:
    #   "s" = fused stt on DVE, "ad"/"pd" = cc-product on ACT/Pool + DVE tt
    CRX_PLANS = [
        dict(pr4=("p", "a", "a", "p"), ad=("fa", "s")),
        dict(pr4=("a", "p", "p", "a"), ad=("s", "fp")),
        dict(pr4=("p", "a", "a", "p"), ad=("s", "fa")),
        dict(pr4=("a", "p", "p", "a"), ad=("fp", "s")),
    ]
    counters = {"rot": 0, "crx": 0}

    with (
        tc.tile_pool(name="const", bufs=1) as cpool,
        tc.tile_pool(name="work", bufs=2) as pool,
        tc.tile_pool(name="tmps", bufs=1) as tpool,
    ):
        cf_t = cpool.tile([P, 16], F32)
        nc.sync.dma_start(cf_t[:, 0 : cf_d.shape[1]], cf_d[:])
        pi2 = cpool.tile([P, 1], F32)
        nc.gpsimd.memset(pi2[:], PI_2)
        pi2c = pi2[:, 0:1]

        for t in range(n_tiles):
            r0, r1 = t * P, (t + 1) * P
            # ---- loads ----
            par = pool.tile([P, 300], F32, tag="par")
            for c in range(3):
                nc.sync.dma_start(par[:, 100 * c : 100 * (c + 1)], par_d[c][r0:r1, :])
            ang = pool.tile([P, NQ], F32, tag="ang")
            nc.sync.dma_start(ang[:], ang_d[r0:r1, :])
            dth = pool.tile([P, 1], F32, tag="dth")
            nc.sync.dma_start(dth[:], dth_d[r0:r1, :])

            # ---- trig ----
            # ScalarE Sin covers [-pi, pi]; quarter angles:
            # u = sin(h/2), w = cos(h/2); sin(h)=2uw, cos(h)=1-2u^2.
            ch = pool.tile([P, 300], F32, tag="ch")
            sh = pool.tile([P, 300], F32, tag="sh")
            nsh = pool.tile([P, 300], F32, tag="nsh")
            trA = pool.tile([P, 100], F32, tag="trA")
            trB = pool.tile([P, 100], F32, tag="trB")

            def emit_trig(dst_s, dst_c, src, scale, scrA, scrB):
                nc.scalar.activation(dst_s, src, ACTF.Sin, scale=scale)
                nc.scalar.activation(dst_c, src, ACTF.Sin, scale=scale, bias=pi2c)
                tt_p(scrA, dst_s, dst_c, op=AOT.mult)
                tt_p(scrB, dst_s, dst_s, op=AOT.mult)
                nc.gpsimd.tensor_scalar_mul(dst_s, scrA, 2.0)
                nc.gpsimd.tensor_scalar(dst_c, scrB, -2.0, 1.0, op0=AOT.mult, op1=AOT.add)

            for c in range(3):
                src = par[:, 100 * c : 100 * (c + 1)]
                dst_s = sh[:, 100 * c : 100 * (c + 1)]
                dst_c = ch[:, 100 * c : 100 * (c + 1)]
                if c == DIAG:
                    emit_trig(dst_s, dst_c, src, 0.25, trA[:], trB[:])
                else:
                    emit_trig(dst_s, dst_c, src, dth[:, 0:1], trA[:], trB[:])
                    # fix CRX cols (30-49, 80-99): no dt factor
                    lx = lambda ap: ap.rearrange("p (l x) -> p l x", l=2, x=50)[:, :, 30:50]
                    emit_trig(
                        lx(dst_s), lx(dst_c), lx(src), 0.25,
                        trA[:, 0:40].rearrange("p (l x) -> p l x", l=2, x=20),
                        trB[:, 0:40].rearrange("p (l x) -> p l x", l=2, x=20),
                    )
            nc.vector.tensor_scalar_mul(nsh[:], sh[:], -1.0)

            angc = pool.tile([P, NQ], F32, tag="angc")
            angs = pool.tile([P, NQ], F32, tag="angs")
            emit_trig(angs[:], angc[:], ang[:], 0.25, trA[:, 0:NQ], trB[:, 0:NQ])
            a3c = pool.tile([P, 30], F32, tag="a3c")
            a3s = pool.tile([P, 30], F32, tag="a3s")
            nc.scalar.copy(a3c[:, 0:10], angc[:])
            nc.scalar.copy(a3c[:, 10:20], angc[:, ::-1])
            nc.scalar.copy(a3c[:, 20:30], angc[:])
            nc.scalar.copy(a3s[:, 0:10], angs[:])
            nc.scalar.copy(a3s[:, 10:20], angs[:, ::-1])
            nc.scalar.copy(a3s[:, 20:30], angs[:])

            # ---- u-coefficients per layer: p,q,nq,r,nr,s,ns (128,30) ----
            ch3 = ch[:].rearrange("p (c x) -> p c x", c=3, x=100)
            sh3 = sh[:].rearrange("p (c x) -> p c x", c=3, x=100)
            U = []
            m1 = pool.tile([P, 30], F32, tag="m1")
            m2 = pool.tile([P, 30], F32, tag="m2")
            m3 = pool.tile([P, 30], F32, tag="m3")
            m4 = pool.tile([P, 30], F32, tag="m4")
            w1 = pool.tile([P, 30], F32, tag="w1")
            w2 = pool.tile([P, 30], F32, tag="w2")
            V = lambda tl: tl[:].rearrange("p (c g) -> p c g", c=3, g=10)
            for L in range(2):
                ca = ch3[:, :, 50 * L : 50 * L + 30 : 3]
                cb = ch3[:, :, 50 * L + 1 : 50 * L + 30 : 3]
                cg = ch3[:, :, 50 * L + 2 : 50 * L + 30 : 3]
                sa = sh3[:, :, 50 * L : 50 * L + 30 : 3]
                sb = sh3[:, :, 50 * L + 1 : 50 * L + 30 : 3]
                sg = sh3[:, :, 50 * L + 2 : 50 * L + 30 : 3]
                u = {
                    k: pool.tile([P, 30], F32, tag=f"u{k}{L}", name=f"u{k}{L}")
                    for k in ("p", "q", "nq", "r", "nr", "s", "ns")
                }
                tt_p(V(m1), cb, ca, op=AOT.mult)
                tt_p(V(m2), sb, sa, op=AOT.mult)
                tt_p(V(m3), sb, ca, op=AOT.mult)
                tt_p(V(m4), cb, sa, op=AOT.mult)
                tt_p(V(w1), cg, V(m1), op=AOT.mult)
                tt_p(V(w2), sg, V(m2), op=AOT.mult)
                tt_p(V(u["p"]), V(w1), V(w2), op=AOT.add)
                tt_p(V(w1), cg, V(m2), op=AOT.mult)
                tt_p(V(w2), sg, V(m1), op=AOT.mult)
                tt_p(V(u["q"]), V(w1), V(w2), op=AOT.subtract)
                tt_p(V(w1), cg, V(m3), op=AOT.mult)
                tt_p(V(w2), sg, V(m4), op=AOT.mult)
                tt_p(V(u["nr"]), V(w1), V(w2), op=AOT.add)
                tt_p(V(w1), sg, V(m3), op=AOT.mult)
                tt_p(V(w2), cg, V(m4), op=AOT.mult)
                tt_p(V(u["s"]), V(w1), V(w2), op=AOT.subtract)
                nc.gpsimd.tensor_scalar_mul(u["nq"][:], u["q"][:], -1.0)
                nc.gpsimd.tensor_scalar_mul(u["r"][:], u["nr"][:], -1.0)
                nc.gpsimd.tensor_scalar_mul(u["ns"][:], u["s"][:], -1.0)
                U.append(u)

            # ---- v vectors: layer-0 rotations folded into init ----
            u0 = U[0]
            v0r = pool.tile([P, 30], F32, tag="v0r")
            v0i = pool.tile([P, 30], F32, tag="v0i")
            v1r = pool.tile([P, 30], F32, tag="v1r")
            v1i = pool.tile([P, 30], F32, tag="v1i")
            for dst, t1, t2 in (
                (v0r, ("p", a3c), ("r", a3s)),
                (v0i, ("q", a3c), ("s", a3s)),
                (v1r, ("nr", a3c), ("p", a3s)),
                (v1i, ("s", a3c), ("nq", a3s)),
            ):
                tt_p(w1[:], u0[t1[0]][:], t1[1][:], op=AOT.mult)
                tt_p(w2[:], u0[t2[0]][:], t2[1][:], op=AOT.mult)
                tt_p(dst[:], w1[:], w2[:], op=AOT.add)

            # ---- per-circuit state (f16) + scratch ----
            st = [pool.tile([P, 2 * DIM], F16, tag=f"st{c}", name=f"st{c}") for c in range(3)]
            tmp = [
                [
                    pool.tile([P, 2 * DIM], F16, tag=f"tmp{c}_{k}", name=f"tmp{c}_{k}")
                    for k in range(4)
                ]
                for c in range(3)
            ]
            ab = [
                [pool.tile([P, 32], F16, tag=f"ab{c}_{k}", name=f"ab{c}_{k}") for k in range(8)]
                for c in range(3)
            ]
            arXs = [pool.tile([P, DIM], F16, tag=f"arX{c}", name=f"arX{c}") for c in range(3)]
            aiXs = [pool.tile([P, DIM], F16, tag=f"aiX{c}", name=f"aiX{c}") for c in range(3)]

            def expand(c, bufs, wires, col_of):
                """Log-doubling product build over `wires` into bufs (r,i,r2,i2)."""
                br, bi, br2, bi2 = bufs
                j0 = col_of(wires[0])
                nc.vector.tensor_copy(br[:, 0:1], v0r[:, j0 : j0 + 1])
                nc.vector.tensor_copy(br[:, 1:2], v1r[:, j0 : j0 + 1])
                nc.vector.tensor_copy(bi[:, 0:1], v0i[:, j0 : j0 + 1])
                nc.vector.tensor_copy(bi[:, 1:2], v1i[:, j0 : j0 + 1])
                width = 2
                cur_r, cur_i, oth_r, oth_i = br, bi, br2, bi2
                for w in wires[1:]:
                    j = col_of(w)
                    c0r, c0i = v0r[:, j : j + 1], v0i[:, j : j + 1]
                    c1r, c1i = v1r[:, j : j + 1], v1i[:, j : j + 1]
                    old_r, old_i = cur_r[:, 0:width], cur_i[:, 0:width]
                    nw = 2 * width
                    nr_v = oth_r[:, 0:nw].rearrange("p (w t) -> p w t", w=width, t=2)
                    ni_v = oth_i[:, 0:nw].rearrange("p (w t) -> p w t", w=width, t=2)
                    tt0 = tmp[c][0][:, 0:width]
                    tt1 = tmp[c][1][:, 0:width]
                    tt2 = tmp[c][2][:, 0:width]
                    tt3 = tmp[c][3][:, 0:width]
                    # (r+ii)(cr+ici): re = r*cr - i*ci ; im = r*ci + i*cr
                    ts_p(tt0, old_i, c0i)
                    nc.vector.scalar_tensor_tensor(
                        nr_v[:, :, 0], old_r, c0r, tt0, op0=AOT.mult, op1=AOT.subtract
                    )
                    ts_p(tt1, old_i, c0r)
                    stt_d(ni_v[:, :, 0], old_r, c0i, tt1)
                    ts_p(tt2, old_i, c1i)
                    nc.vector.scalar_tensor_tensor(
                        nr_v[:, :, 1], old_r, c1r, tt2, op0=AOT.mult, op1=AOT.subtract
                    )
                    ts_p(tt3, old_i, c1r)
                    stt_d(ni_v[:, :, 1], old_r, c1i, tt3)
                    cur_r, oth_r = oth_r, cur_r
                    cur_i, oth_i = oth_i, cur_i
                    width = nw
                return cur_r, cur_i

            def emit_rot(c, stt_c, w, u, j):
                """General SU(2) gate on wire w, full state.

                4 whole-state (2048) scalar products + 9 f16 tt add/subs:
                  a0' = p*a0 + r*a1 - (q*a0i + s*a1i | -(q*a0r + s*a1r))
                  a1' = p*a1 - r*a0 + (q*a1i - s*a0i | s*a0r - q*a1r)
                """
                plan = ROT_PLANS[counters["rot"] % len(ROT_PLANS)]
                counters["rot"] += 1
                sp = u["p"][:, j : j + 1]
                sq = u["q"][:, j : j + 1]
                sr = u["r"][:, j : j + 1]
                ss = u["s"][:, j : j + 1]
                re, im = stt_c[:, 0:DIM], stt_c[:, DIM : 2 * DIM]
                inner = 1 << (NQ - 1 - w)
                outer = HD // inner
                m = 2 * outer
                fm = stt_c[:].rearrange("p (m t i) -> p m t i", m=m, t=2, i=inner)
                a0m = fm[:, :, 0, :]
                a1m = fm[:, :, 1, :]
                a0r, a1r = _wire_views(re, w)
                a0i, a1i = _wire_views(im, w)
                TW, RP, QF, SF = tmp[c][0:4]
                tv = lambda tp, tbit: tp[:].rearrange(
                    "p (m t i) -> p m t i", m=m, t=2, i=inner
                )[:, :, tbit, :]
                pv = lambda tp, off, tbit: tp[:, off : off + DIM].rearrange(
                    "p (o t i) -> p o t i", o=outer, t=2, i=inner
                )[:, :, tbit, :]
                hv = lambda tp, h: tp[:, h * HD : (h + 1) * HD].rearrange(
                    "p (o i) -> p o i", o=outer, i=inner
                )
                E = [TS[k] for k in plan]
                # whole-state products (2048 each)
                E[0](TW[:], stt_c[:], sp)
                E[1](RP[:], stt_c[:], sr)
                E[2](QF[:], stt_c[:], sq)   # [q*re | q*im]
                E[3](SF[:], stt_c[:], ss)   # [s*re | s*im]
                # aligned into state (reads TW/RP only)
                tt_d(a0m, tv(TW, 0), tv(RP, 1), op=AOT.add)
                tt_d(a1m, tv(TW, 1), tv(RP, 0), op=AOT.subtract)
                # swapped partials into TW halves (TW dead after aligned adds)
                # T1n = s*a1i + q*a0i ; T1i = q*a0r + s*a1r
                # T3r = q*a1i - s*a0i ; T3i = s*a0r - q*a1r
                tt_d(hv(TW, 0), pv(SF, DIM, 1), pv(QF, DIM, 0), op=AOT.add)
                tt_d(hv(TW, 1), pv(QF, 0, 0), pv(SF, 0, 1), op=AOT.add)
                tt_d(hv(TW, 2), pv(QF, DIM, 1), pv(SF, DIM, 0), op=AOT.subtract)
                tt_d(hv(TW, 3), pv(SF, 0, 0), pv(QF, 0, 1), op=AOT.subtract)
                # finals: a0 per-half (sign differs), a1 merged
                tt_d(a0r, a0r, hv(TW, 0), op=AOT.subtract)
                tt_d(a0i, a0i, hv(TW, 1), op=AOT.add)
                t3m = TW[:, DIM : 2 * DIM].rearrange("p (m i) -> p m i", m=m, i=inner)
                tt_d(a1m, a1m, t3m, op=AOT.add)

            def emit_crx(c, stt_c, ctrl, tgt, col):
                """CRX via baseline-proven views: 4 quarter products + 2
                merged fused adds (stt on DVE)."""
                plan = CRX_PLANS[counters["crx"] % len(CRX_PLANS)]
                counters["crx"] += 1
                cc = ch[:, col : col + 1]
                ss = sh[:, col : col + 1]
                ns = nsh[:, col : col + 1]
                re, im = stt_c[:, 0:DIM], stt_c[:, DIM : 2 * DIM]
                q0r, q1r = _qviews2(re, ctrl, tgt)
                q0i, q1i = _qviews2(im, ctrl, tgt)
                hi, lo = min(ctrl, tgt), max(ctrl, tgt)
                t23 = tmp[c][2][:, 0:HD]
                ee = tmp[c][2][:, HD:DIM]
                E = [TS[k] for k in plan["pr4"]]
                if lo - hi == 1:
                    a = 1 << hi
                    z = 1 << (NQ - 2 - hi)
                    ma = 2 * a
                    fm = stt_c[:].rearrange(
                        "p (ma x y z) -> p ma x y z", ma=ma, x=2, y=2, z=z
                    )
                    if ctrl < tgt:
                        q0m = fm[:, :, 1, 0, :]
                        q1m = fm[:, :, 1, 1, :]
                    else:
                        q0m = fm[:, :, 0, 1, :]
                        q1m = fm[:, :, 1, 1, :]
                    pvm = lambda tp: tp.rearrange("p (ma z) -> p ma z", ma=ma, z=z)
                    hv2 = lambda tp, h: tp[:, h * QT : (h + 1) * QT].rearrange(
                        "p (a z) -> p a z", a=a, z=z
                    )
                    E[0](hv2(t23, 0), q0i, ss)
                    E[1](hv2(t23, 1), q0r, ns)
                    E[2](hv2(ee, 0), q1i, ss)
                    E[3](hv2(ee, 1), q1r, ns)

                    def fadd2(qm, addend, mode, off):
                        if mode == "s":
                            stt_d(qm, qm, cc, addend)
                            return
                        pbv = tmp[c][3][:, off : off + HD].rearrange(
                            "p (ma z) -> p ma z", ma=ma, z=z
                        )
                        TS["a" if mode == "fa" else "p"](pbv, qm, cc)
                        tt_d(qm, pbv, addend, op=AOT.add)

                    fadd2(q0m, pvm(ee), plan["ad"][0], 0)
                    fadd2(q1m, pvm(t23), plan["ad"][1], HD)
                else:
                    t0 = t23[:, 0:QT]
                    t1 = t23[:, QT:HD]
                    t2 = ee[:, 0:QT]
                    t3 = ee[:, QT:HD]
                    E[0](t0, q0r, cc)
                    E[1](t1, q0i, cc)
                    E[2](t2, q0i, ss)
                    E[3](t3, q0r, ns)
                    stt_d(q0r, q1i, ss, t0)
                    stt_d(q0i, q1r, ns, t1)
                    stt_d(q1r, q1r, cc, t2)
                    stt_d(q1i, q1i, cc, t3)

            col_ofs = [
                (lambda w, c=c: 10 * c + (w if c != BWD else NQ - 1 - w))
                for c in range(3)
            ]
            abv = {}
            for c in range(3):
                abv[c] = expand(c, ab[c][0:4], list(range(5)), col_ofs[c])
            bbv = {}
            for c in range(3):
                bbv[c] = expand(c, ab[c][4:8], list(range(5, NQ)), col_ofs[c])
            for c in range(3):
                ar, ai = abv[c]
                br_, bi_ = bbv[c]
                sre = st[c][:, 0:DIM].rearrange("p (i j) -> p i j", i=32, j=32)
                sim_ = st[c][:, DIM : 2 * DIM].rearrange("p (i j) -> p i j", i=32, j=32)
                arXv = arXs[c][:].rearrange("p (i j) -> p i j", i=32, j=32)
                aiXv = aiXs[c][:].rearrange("p (i j) -> p i j", i=32, j=32)
                arb = ar[:].broadcast_to([P, 32, 32])
                aib = ai[:].broadcast_to([P, 32, 32])
                brb = br_[:].broadcast_to([P, 32, 32]).transpose([0, 2, 1])
                bib = bi_[:].broadcast_to([P, 32, 32]).transpose([0, 2, 1])
                # materialize the stride-0-innermost operands once -> packed TTs
                nc.vector.tensor_copy(arXv, arb)
                nc.vector.tensor_copy(aiXv, aib)
                tt_d(sre, arXv, brb, op=AOT.mult)
                tt_p(sim_, aiXv, bib, op=AOT.mult)
                tt_d(sre, sre, sim_, op=AOT.subtract)
                tt_d(sim_, arXv, bib, op=AOT.mult)
                scrv = tmp[c][2][:, 0:DIM].rearrange("p (i j) -> p i j", i=32, j=32)
                tt_p(scrv, aiXv, brb, op=AOT.mult)
                tt_d(sim_, sim_, scrv, op=AOT.add)

            # gates, interleaved round-robin across circuits so the three
            # independent chains pack the engines
            ring0 = [_ring_gates(c, 0) for c in range(3)]
            ring1 = [_ring_gates(c, 1) for c in range(3)]
            for k in range(2 * NQ):
                for c in range(3):
                    ctrl, tgt, col = ring0[c][k]
                    emit_crx(c, st[c], ctrl, tgt, col)
            for g in range(NQ):
                for c in range(3):
                    emit_rot(c, st[c], _wire(c, g), U[1], 10 * c + g)
            for k in range(2 * NQ):
                for c in range(3):
                    ctrl, tgt, col = ring1[c][k]
                    emit_crx(c, st[c], ctrl, tgt, col)

            # ---- combine: acc = c1*psi1 + c2*psi2 + c3*psi3 (f16) ----
            acc = pool.tile([P, 2 * DIM], F16, tag="acc")
            s1t = pool.tile([P, 2 * DIM], F16, tag="s1t")
            cfc = lambda k: cf_t[:, k : k + 1]
            # cf cols: [c1r, c1i, nc1i, c2r, c2i, nc2i, c3r, c3i, nc3i]
            for k in range(3):
                stk = st[k]
                swapped = stk[:].rearrange("p (pl x) -> p pl x", pl=2, x=DIM)[:, ::-1, :]
                if k == 0:
                    ts_d(acc[:], stk[:], cfc(0))
                    ts_d(s1t[:], stk[:], cfc(1))
                else:
                    stt_d(acc[:], stk[:], cfc(3 * k), acc[:])
                    ts_d(s1t[:], stk[:], cfc(3 * k + 1))
                # acc_re -= c_i*st_im ; acc_im += c_i*st_re
                tt_d(acc[:, 0:DIM], acc[:, 0:DIM], s1t[:, DIM : 2 * DIM], op=AOT.subtract)
                tt_d(acc[:, DIM : 2 * DIM], acc[:, DIM : 2 * DIM], s1t[:, 0:DIM], op=AOT.add)

            # ---- measure ----
            xw = pool.tile([P, NQ], F32, tag="xw")
            yw = pool.tile([P, NQ], F32, tag="yw")
            zw = pool.tile([P, NQ], F32, tag="zw")
            scol = pool.tile([P, 8], F32, tag="scol")
            mscr = pool.tile([P, DIM], F16, tag="mscr")
            mx = pool.tile([P, DIM], F16, tag="mx")
            mY = pool.tile([P, 2 * DIM], F16, tag="mY")

            # S = sum |acc|^2  (proven stt-accumulate pattern)
            nc.vector.scalar_tensor_tensor(
                mY[:], acc[:], 0.0, acc[:], op0=AOT.bypass, op1=AOT.mult,
                accum_out=scol[:, 0:1],
            )
            # mY = [acc_im | -acc_re]  for the Y cross term
            nc.vector.tensor_copy(mY[:, 0:DIM], acc[:, DIM : 2 * DIM])
            ts_d(mY[:, DIM : 2 * DIM], acc[:, 0:DIM], -1.0)

            for w in range(NQ):
                inner = 1 << (NQ - 1 - w)
                m = 2 * (HD // inner)
                fv = acc[:].rearrange("p (m t i) -> p m t i", m=m, t=2, i=inner)
                p0 = fv[:, :, 0, :]
                p1 = fv[:, :, 1, :]
                fy = mY[:].rearrange("p (m t i) -> p m t i", m=m, t=2, i=inner)
                p1y = fy[:, :, 1, :]
                ms = mscr[:].rearrange("p (m i) -> p m i", m=m, i=inner)
                mxv = mx[:].rearrange("p (m i) -> p m i", m=m, i=inner)
                tt_d(mxv, p0, p1, op=AOT.add)
                nc.scalar.activation(
                    mxv, mxv, ACTF.Square, accum_out=xw[:, w : w + 1]
                )
                nc.vector.scalar_tensor_tensor(
                    ms, p0, 0.0, p1y, op0=AOT.bypass, op1=AOT.mult,
                    accum_out=yw[:, w : w + 1],
                )
                nc.scalar.activation(
                    ms, p1, ACTF.Square, accum_out=zw[:, w : w + 1]
                )

            # inv = 1/(S + 1e-9); s2 = 2*inv; sn2 = -2*inv; sz = S*inv
            nc.vector.tensor_scalar(
                scol[:, 1:2], scol[:, 0:1], 1e-9, None, op0=AOT.add
            )
            nc.vector.reciprocal(scol[:, 2:3], scol[:, 1:2])
            nc.vector.tensor_scalar(scol[:, 3:4], scol[:, 2:3], 2.0, None, op0=AOT.mult)
            nc.vector.tensor_scalar(scol[:, 4:5], scol[:, 2:3], -2.0, None, op0=AOT.mult)
            tt_d(scol[:, 5:6], scol[:, 0:1], scol[:, 2:3], op=AOT.mult)
            nc.vector.tensor_scalar(scol[:, 6:7], scol[:, 5:6], -1.0, None, op0=AOT.mult)

            out30 = pool.tile([P, 30], F32, tag="out30")
            nc.vector.tensor_scalar(
                out30[:, 0:10], xw[:], scol[:, 2:3], scol[:, 6:7],
                op0=AOT.mult, op1=AOT.add,
            )
            ts_d(out30[:, 10:20], yw[:], scol[:, 3:4])
            szb = scol[:, 5:6].broadcast_to([P, 1, NQ])
            nc.vector.scalar_tensor_tensor(
                out30[:, 20:30].unsqueeze(1), zw[:].unsqueeze(1), scol[:, 4:5], szb,
                op0=AOT.mult, op1=AOT.add,
            )
            nc.sync.dma_start(out_d[r0:r1, :], out30[:])


def build_nc(n_tiles=NT, b_core=None):
    if b_core is None:
        b_core = n_tiles * P
    nc = bacc.Bacc("TRN2", target_bir_lowering=False)
    ins = {
        "input_angles": nc.dram_tensor("input_angles", [b_core, NQ], F32, kind="ExternalInput")[:],
        "forward_params": nc.dram_tensor("forward_params", [b_core, 100], F32, kind="ExternalInput")[:],
        "backward_params": nc.dram_tensor("backward_params", [b_core, 100], F32, kind="ExternalInput")[:],
        "diagonal_params": nc.dram_tensor("diagonal_params", [b_core, 100], F32, kind="ExternalInput")[:],
        "dth": nc.dram_tensor("dth", [b_core, 1], F32, kind="ExternalInput")[:],
        "cf": nc.dram_tensor("cf", [P, 9], F32, kind="ExternalInput")[:],
    }
    outs = {"out": nc.dram_tensor("out", [b_core, 30], F32, kind="ExternalOutput")[:]}
    with tile.TileContext(nc) as tc:
        emit_core_kernel(nc, tc, ins, outs, n_tiles=n_tiles)
    nc.compile()
    return nc


_NC_CACHE = {}


def _get_nc(n_tiles=NT):
    if n_tiles not in _NC_CACHE:
        _NC_CACHE[n_tiles] = build_nc(n_tiles)
    return _NC_CACHE[n_tiles]


def make_host_inputs(input_angles, forward_params, backward_params, diagonal_params,
                     dt_scale, alpha_real, alpha_imag, beta_real, beta_imag,
                     gamma_real, gamma_imag):
    """Host-side scalar prep shared by kernel() and tests."""
    al = complex(float(alpha_real), float(alpha_imag))
    be = complex(float(beta_real), float(beta_imag))
    ga = complex(float(gamma_real), float(gamma_imag))
    n = np.sqrt(abs(al) ** 2 + abs(be) ** 2 + abs(ga) ** 2 + 1e-9)
    cs = [al / n, be / n, ga / n]
    row = []
    for ck in cs:
        row += [ck.real, ck.imag, -ck.imag]
    cf = np.tile(np.asarray(row, np.float32), (P, 1))
    dth = (0.25 * np.asarray(dt_scale, np.float32)).reshape(-1, 1)
    return cf, dth


def kernel(**inputs):
    from concourse.bass_utils import run_bass_kernel_spmd

    cf, dth = make_host_inputs(**inputs)
    ang = np.ascontiguousarray(np.asarray(inputs["input_angles"], np.float32))
    pf = np.ascontiguousarray(np.asarray(inputs["forward_params"], np.float32))
    pb = np.ascontiguousarray(np.asarray(inputs["backward_params"], np.float32))
    pd = np.ascontiguousarray(np.asarray(inputs["diagonal_params"], np.float32))

    nc = _get_nc(NT)
    in_maps = []
    for c in range(N_CORES):
        r0, r1 = c * B_CORE, (c + 1) * B_CORE
        in_maps.append({
            "input_angles": ang[r0:r1],
            "forward_params": pf[r0:r1],
            "backward_params": pb[r0:r1],
            "diagonal_params": pd[r0:r1],
            "dth": np.ascontiguousarray(dth[r0:r1]),
            "cf": cf,
        })
    res = run_bass_kernel_spmd(nc, in_maps, core_ids=list(range(N_CORES)))
    out = np.concatenate([res.results[c]["out"] for c in range(N_CORES)], axis=0)
    return out.astype(np.float32)
